# revision 25
# baseline (speedup 1.0000x reference)
"""GQA MultiHeadAttention (RoPE, causal) Bass/Tile kernel for 8 Trainium2 cores.

Problem: x[2,2048,2048] @ Wq/Wk/Wv -> RoPE -> causal GQA attention -> @ Wo.
D=2048, H=16 heads, G=4 KV groups, HD=128, B=2, S=2048.

Sharding (SPMD, one program, per-core data):
  core c -> batch b=c//4, KV-group g=c%4 (heads 4g..4g+3).
  Each core: QKV projection for its group from x[b]^T, RoPE, 4 heads of
  attention, and a row-shard of the output projection (Wo rows for its
  heads) producing a partial [2048,2048] output. Host sums the 4 partials
  per batch.

On-chip layouts are all "transposed" (feature dim on partitions):
  QT/KT/VT [hd, s]; scores computed as scoresT [k, q]; ctxT [hd, q];
  out-projection uses ctxT slices as stationary to produce natural [s, d].

v2 changes vs the 364us baseline:
  - bf16 on the whole matmul/DMA path (PSUM accumulate stays f32);
    halves HBM traffic and SBUF footprint.
  - softmax denominator off the PE: DVE accumulates exp tiles per head,
    GpSimd partition_all_reduce gives the broadcast denominator, DVE
    reciprocal+scale evicts ctx. Removes 160 PE den matmuls and the 16
    broadcast matmuls (and their head-boundary PE stalls).
  - diagonal score tiles compute only the valid q-range (moving operand
    shrinks 512->128..512); causal mask applies to one 128x128 subblock.
  - off-diagonal k-tiles run first within each head so diagonal partial
    regions accumulate onto an initialized PSUM bank.
"""

import sys

if "/opt/trn_rl_repo" not in sys.path:
    sys.path.insert(0, "/opt/trn_rl_repo")

from contextlib import ExitStack

import numpy as np
import ml_dtypes

import concourse.bass as bass
import concourse.tile as tile
from concourse import bacc, bass_isa, mybir
from concourse.bass_utils import run_bass_kernel_spmd
from concourse.masks import make_identity

F32 = mybir.dt.float32
BF16 = mybir.dt.bfloat16
AF = mybir.ActivationFunctionType
NPBF16 = np.dtype(ml_dtypes.bfloat16)

B, S, D = 2, 2048, 2048
H, G, HD = 16, 4, 128
HPG = H // G          # heads per group = 4
GD = HPG * HD         # group width = 512
P = 128
NCHUNK = 512          # matmul moving free dim
SC = S // NCHUNK      # 4 s-chunks
DT = D // P           # 16 d-tiles
ST = S // P           # 16 s-tiles
SCALE = 1.0 / float(np.sqrt(HD))

_CACHE = {}


def _build():
    nc = bacc.Bacc("TRN2", target_bir_lowering=False, debug=False, num_devices=8)

    # ---- DRAM I/O (per-core shards, bf16 on the wire) ----
    xT = nc.dram_tensor("xT", [D, S], BF16, kind="ExternalInput").ap()
    wq = nc.dram_tensor("wq", [D, GD], BF16, kind="ExternalInput").ap()
    wk = nc.dram_tensor("wk", [D, HD], BF16, kind="ExternalInput").ap()
    wv = nc.dram_tensor("wv", [D, HD], BF16, kind="ExternalInput").ap()
    wo = nc.dram_tensor("wo", [GD, D], BF16, kind="ExternalInput").ap()
    cosT = nc.dram_tensor("cosT", [HD, S], BF16, kind="ExternalInput").ap()
    sinT = nc.dram_tensor("sinT", [HD, S], F32, kind="ExternalInput").ap()
    prot = nc.dram_tensor("prot", [HD, HD], BF16, kind="ExternalInput").ap()
    out = nc.dram_tensor("out", [S, D], BF16, kind="ExternalOutput").ap()

    xT_v = xT.rearrange("(t p) s -> p t s", p=P)          # [128, 16, 2048]
    wq_v = wq.rearrange("(t p) o -> p t o", p=P)          # [128, 16, 512]
    wk_v = wk.rearrange("(t p) o -> p t o", p=P)          # [128, 16, 128]
    wv_v = wv.rearrange("(t p) o -> p t o", p=P)
    wo_v = wo.rearrange("(h p) d -> p h d", p=P)          # [128, 4, 2048]
    out_v = out.rearrange("(t p) d -> t p d", p=P)        # [16, 128, 2048]

    with tile.TileContext(nc) as tc:
        with ExitStack() as ctx:
            pers = ctx.enter_context(tc.tile_pool(name="pers", bufs=1))
            psum = ctx.enter_context(tc.tile_pool(name="psum", bufs=6, space="PSUM"))
            xpool = ctx.enter_context(tc.tile_pool(name="xpool", bufs=10))
            spool = ctx.enter_context(tc.tile_pool(name="spool", bufs=4))
            epool = ctx.enter_context(tc.tile_pool(name="epool", bufs=8))
            dpool = ctx.enter_context(tc.tile_pool(name="dpool", bufs=3))
            wopool = ctx.enter_context(tc.tile_pool(name="wopool", bufs=3))
            cpool = ctx.enter_context(tc.tile_pool(name="cpool", bufs=2))
            cspool = ctx.enter_context(tc.tile_pool(name="cspool", bufs=3))
            evpool = ctx.enter_context(tc.tile_pool(name="evpool", bufs=6))
            opool = ctx.enter_context(tc.tile_pool(name="opool", bufs=4))

            _bank_n = [0]

            def bank():
                _bank_n[0] += 1
                return psum.tile([P, NCHUNK], F32, tag="bank",
                                 name=f"bank{_bank_n[0]}")

            # ---- persistent tiles ----
            wq_t = pers.tile([P, DT, GD], BF16, tag="wq")
            wk_t = pers.tile([P, DT, HD], BF16, tag="wk")
            wv_t = pers.tile([P, DT, HD], BF16, tag="wv")
            cos_t = pers.tile([P, S], BF16, tag="cos")
            sin_t = pers.tile([P, S], F32, tag="sin")
            prot_t = pers.tile([P, HD], BF16, tag="prot")
            ident = pers.tile([P, P], BF16, tag="ident")
            masksb = pers.tile([P, P], BF16, tag="masksb")    # 0 / -1e4 tri
            onescol = pers.tile([P, 1], BF16, tag="onescol")
            onesrow = pers.tile([1, P], BF16, tag="onesrow")
            qf = pers.tile([P, HPG, S], BF16, tag="qf")       # roped Q^T, 4 heads
            kf = pers.tile([P, S], BF16, tag="kf")            # roped K^T
            vnat = pers.tile([P, ST, HD], BF16, tag="vnat")   # V natural [s, hd]

            # Batched x loads: 2-dt pieces for the first s-chunk (so the
            # first matmuls start after ~0.6 MB of DMA), whole 8-dt groups
            # prefetched one chunk ahead for the rest. Few dma_starts keeps
            # the SP sequencer's per-issue DGE overhead off the critical path.
            xgroups = {}

            def stage_x(sc, pieces, tag, bufs):
                s0 = sc * NCHUNK
                lst = []
                for d0, d1 in pieces:
                    xg = xpool.tile([P, d1 - d0, NCHUNK], BF16, tag=tag,
                                    bufs=bufs, name=f"xg{sc}_{d0}")
                    nc.sync.dma_start(xg[:], xT_v[:, d0:d1, s0:s0 + NCHUNK])
                    lst.append((d0, d1, xg))
                xgroups.setdefault(sc, []).extend(lst)

            def xslice(sc, dt):
                for d0, d1, xg in xgroups[sc]:
                    if d0 <= dt < d1:
                        return xg[:, dt - d0, :]
                raise KeyError((sc, dt))

            # chunk-0 x pieces interleaved with per-piece weight loads;
            # the first two pieces are single-dt so the first matmul starts
            # after ~0.3 MB of DMA
            pieces0 = [(0, 1), (1, 2)] + [(d, d + 2) for d in range(2, DT, 2)]
            for d0, d1 in pieces0:
                nc.sync.dma_start(wq_t[:, d0:d1, :], wq_v[:, d0:d1, :])
                stage_x(0, [(d0, d1)], "xg0", 9)
            nc.sync.dma_start(wk_t[:], wk_v[:])
            nc.sync.dma_start(wv_t[:], wv_v[:])
            nc.sync.dma_start(cos_t[:], cosT[:])
            nc.sync.dma_start(sin_t[:], sinT[:])
            nc.sync.dma_start(prot_t[:], prot[:])
            make_identity(nc, ident[:])
            # additive causal mask for diagonal 128x128 subblocks:
            # masksb[r, c] = 0 where c >= r (keep), -1e4 where c < r, so a
            # single PE matmul (stat=ident) accumulates it into scores PSUM
            # and exp underflows the masked slots to 0. Keeps GpSimd out of
            # the scores->exp->ctx chain entirely.
            nc.gpsimd.memset(onescol[:], 1.0)
            nc.gpsimd.memset(onesrow[:], 1.0)
            nc.gpsimd.memset(masksb[:], 0.0)
            nc.gpsimd.affine_select(
                out=masksb[:], in_=masksb[:],
                compare_op=mybir.AluOpType.is_ge,
                fill=-1.0e4, base=0, channel_multiplier=-1,
                pattern=[[1, P]],
            )

            # ================= Phase A: QKV projection + RoPE + V^T -> V ====
            def rope(dst, src_sb, sc):
                """dst[128,512] bf16 slice = rope(src_sb [128,512] bf16)."""
                cs = cos_t[:, sc * NCHUNK:(sc + 1) * NCHUNK]
                sn = sin_t[:, sc * NCHUNK:(sc + 1) * NCHUNK]
                rotps = bank()
                nc.tensor.matmul(rotps[:], prot_t[:], src_sb, start=True, stop=True)
                t1 = spool.tile([P, NCHUNK], BF16, tag="t1")
                nc.vector.tensor_mul(t1[:], rotps[:], sn)
                nc.vector.tensor_mul(dst, src_sb, cs)
                nc.vector.tensor_add(dst, dst, t1[:])

            def q_proj(sc, dts):
                """Q projection matmuls for s-chunk sc over d-tiles dts."""
                for dt in dts:
                    xt = xslice(sc, dt)
                    for h in range(HPG):
                        nc.tensor.matmul(
                            _qacc[sc][h][:], wq_t[:, dt, h * HD:(h + 1) * HD],
                            xt, start=dt == 0, stop=dt == DT - 1)

            def kv_proj(sc, dts):
                kps, vps = _kvacc[sc]
                for dt in dts:
                    xt = xslice(sc, dt)
                    nc.tensor.matmul(kps[:], wk_t[:, dt, :], xt,
                                     start=dt == 0, stop=dt == DT - 1)
                    nc.tensor.matmul(vps[:], wv_t[:, dt, :], xt,
                                     start=dt == 0, stop=dt == DT - 1)

            def q_evict(sc):
                """Q PSUM -> SBUF (frees the 4 Q banks before the KV pass)."""
                sbs = []
                for h in range(HPG):
                    qsb = evpool.tile([P, NCHUNK], BF16, tag="ev",
                                      name=f"qsb{sc}_{h}")
                    if h % 2 == 0:
                        nc.scalar.copy(qsb[:], _qacc[sc][h][:])
                    else:
                        nc.vector.tensor_copy(qsb[:], _qacc[sc][h][:])
                    sbs.append(qsb[:])
                return sbs

            def q_tail_steps(sc, sbs):
                """Q RoPE; interleaved into the same chunk's KV pass."""
                s0 = sc * NCHUNK
                for h in range(HPG):
                    rope(qf[:, h, s0:s0 + NCHUNK], sbs[h], sc)
                    yield

            def kv_evict(sc):
                kps, vps = _kvacc[sc]
                ksb = evpool.tile([P, NCHUNK], BF16, tag="ev", name=f"ksb{sc}")
                nc.vector.tensor_copy(ksb[:], kps[:])
                vsb = evpool.tile([P, NCHUNK], BF16, tag="ev", name=f"vsb{sc}")
                nc.scalar.copy(vsb[:], vps[:])
                return ksb, vsb

            def kv_tail_steps(sc, ksb, vsb):
                """K RoPE + V^T transpose; interleaved into the next chunk's
                Q pass (or, for the last chunk, into q-chunk 0's attention —
                kf/vnat of chunk sc are only read by q-chunks >= sc)."""
                rope(kf[:, sc * NCHUNK:(sc + 1) * NCHUNK], ksb[:], sc)
                yield
                for j in range(4):
                    _bank_n[0] += 1
                    tps = psum.tile([P, P], BF16, tag="bank",
                                    name=f"tbank{_bank_n[0]}")
                    nc.tensor.transpose(
                        tps[:], vsb[:, j * P:(j + 1) * P], ident[:])
                    nc.vector.tensor_copy(vnat[:, sc * 4 + j, :], tps[:])
                    yield

            # ---- Phase B/C: attention per (q-chunk, head) + out-projection
            LOOKAHEAD = 6     # tiles of scores->ctx pipeline lag
            NORM_DELAY = 8    # tiles after head end before recip/mul emission

            def attention_steps(qc, ctxq):
                """Generator emitting attention for q-chunk qc one score-unit
                at a time. Off-diagonal k-tiles are processed in PAIRS that
                share a 2-bank PSUM tile and a single exp instruction
                (amortizes the Activation engine's per-op overhead — Act is
                the throughput limiter of this phase). Diagonal tiles run
                singly with the moving operand shrunk to the valid q-range
                and the additive -1e4 triangle matmul'd into scores PSUM.
                The denominator accumulates on DVE into a 2-lane accumulator;
                at head end the ctx bank is evicted to SBUF immediately
                (Scalar) and the reciprocal/normalize (DVE/GpSimd) is
                DEFERRED a few tiles so the partition_all_reduce latency
                (~4us on HW) never head-of-line-blocks either engine."""
                q0 = qc * NCHUNK
                base = 4 * qc
                nki = base + 4
                units = []
                for h in range(HPG):
                    for i in range(0, base, 2):
                        units.append((h, (i, i + 2)))      # off-diag pair
                    for ki in range(base, nki):
                        units.append((h, (ki, ki + 1)))    # diag single
                tile_seq = [(h, ki) for h, u in units for ki in range(*u)]
                ets = {}
                acc2s = {}
                ctxbanks = {}
                csbs = {}
                pending = []   # (tile_index_at_emit, h, denb)

                def qr0_of(ki):
                    return (ki - base) * P if ki >= base else 0

                def do_unit(h, u):
                    a, b = u
                    if b - a == 2:
                        _bank_n[0] += 1
                        sps = psum.tile([P, 2, NCHUNK], F32, tag="bank2",
                                        bufs=1, name=f"b2_{_bank_n[0]}")
                        for j in range(2):
                            nc.tensor.matmul(
                                sps[:, j, :], kf[:, (a + j) * P:(a + j + 1) * P],
                                qf[:, h, q0:q0 + NCHUNK], start=True, stop=True)
                        et = epool.tile([P, 2, NCHUNK], BF16, tag="et2",
                                        bufs=5, name=f"et2_{qc}_{h}_{a}")
                        nc.scalar.activation(et[:], sps[:], AF.Exp, scale=SCALE)
                        if a == 0:
                            acc = dpool.tile([P, 2, NCHUNK], BF16, tag="acc",
                                             name=f"acc{qc}_{h}")
                            acc2s[h] = acc
                            nc.vector.tensor_copy(acc[:], et[:])
                        else:
                            acc = acc2s[h]
                            nc.vector.tensor_add(acc[:], acc[:], et[:])
                        ets[(h, a)] = et[:, 0, :]
                        ets[(h, a + 1)] = et[:, 1, :]
                    else:
                        ki = a
                        qr0 = qr0_of(ki)
                        sps = bank()
                        nc.tensor.matmul(
                            sps[:, qr0:NCHUNK], kf[:, ki * P:(ki + 1) * P],
                            qf[:, h, q0 + qr0:q0 + NCHUNK],
                            start=True, stop=False)
                        nc.tensor.matmul(
                            sps[:, qr0:qr0 + P], ident[:], masksb[:],
                            start=False, stop=True)
                        et = epool.tile([P, NCHUNK], BF16, tag="et",
                                        bufs=6, name=f"et{qc}_{h}_{ki}")
                        nc.scalar.activation(et[:, qr0:NCHUNK],
                                             sps[:, qr0:NCHUNK],
                                             AF.Exp, scale=SCALE)
                        if ki == 0:
                            acc = dpool.tile([P, 2, NCHUNK], BF16, tag="acc",
                                             name=f"acc{qc}_{h}")
                            acc2s[h] = acc
                            nc.vector.tensor_copy(acc[:, 0, :], et[:])
                            nc.vector.memset(acc[:, 1, :], 0.0)
                        else:
                            acc = acc2s[h]
                            nc.vector.tensor_add(
                                acc[:, ki & 1, qr0:NCHUNK],
                                acc[:, ki & 1, qr0:NCHUNK], et[:, qr0:NCHUNK])
                        ets[(h, ki)] = et[:, qr0:NCHUNK]

                def do_ctx(h, ki):
                    qr0 = qr0_of(ki)
                    if ki == 0:
                        ctxbanks[h] = bank()
                    nc.tensor.matmul(ctxbanks[h][:, qr0:NCHUNK],
                                     vnat[:, ki, :], ets.pop((h, ki)),
                                     start=(ki == 0), stop=(ki == nki - 1))
                    if ki == nki - 1:
                        csb = cspool.tile([P, NCHUNK], F32, tag="ctxsb",
                                          name=f"csb{qc}_{h}")
                        nc.scalar.copy(csb[:], ctxbanks[h][:])
                        csbs[h] = csb
                        acc = acc2s.pop(h)
                        den1 = spool.tile([P, NCHUNK], BF16, tag="den1")
                        nc.vector.tensor_add(den1[:], acc[:, 0, :],
                                             acc[:, 1, :])
                        del ctxbanks[h]
                        if qc == SC - 1 and h == HPG - 1:
                            # final head: norm via PE ones-matmuls (~3us
                            # faster than partition_all_reduce and keeps the
                            # DVE queue clear for the bare out-projection)
                            dps = bank()
                            nc.tensor.matmul(dps[0:1, :], onescol[:],
                                             den1[:], start=True, stop=True)
                            rec32 = spool.tile([1, NCHUNK], F32, tag="rec32")
                            nc.vector.reciprocal_approx_fast(rec32[:],
                                                             dps[0:1, :])
                            rec16 = spool.tile([1, NCHUNK], BF16, tag="rec16")
                            nc.vector.tensor_copy(rec16[:], rec32[:])
                            bps = bank()
                            nc.tensor.matmul(bps[:], onesrow[:], rec16[:],
                                             start=True, stop=True)
                            nc.vector.tensor_mul(ctxq[:, h, :],
                                                 csbs.pop(h)[:], bps[:])
                            return None
                        denb = spool.tile([P, NCHUNK], F32, tag="denb")
                        nc.gpsimd.partition_all_reduce(
                            denb[:], den1[:], channels=P,
                            reduce_op=bass_isa.ReduceOp.add)
                        return denb
                    return None

                def flush_norm(h, denb):
                    recb = spool.tile([P, NCHUNK], F32, tag="recb")
                    nc.vector.reciprocal_approx_fast(recb[:], denb[:])
                    nc.vector.tensor_mul(ctxq[:, h, :], csbs.pop(h)[:],
                                         recb[:])

                ti = 0
                emitted = 0
                for h, u in units:
                    do_unit(h, u)
                    emitted += u[1] - u[0]
                    while ti < emitted - LOOKAHEAD:
                        hh, kk = tile_seq[ti]
                        denb = do_ctx(hh, kk)
                        if denb is not None:
                            pending.append((ti, hh, denb))
                        ti += 1
                        if pending and ti - pending[0][0] >= NORM_DELAY:
                            _, ph, pd = pending.pop(0)
                            flush_norm(ph, pd)
                    yield
                while ti < len(tile_seq):
                    hh, kk = tile_seq[ti]
                    denb = do_ctx(hh, kk)
                    if denb is not None:
                        pending.append((ti, hh, denb))
                    ti += 1
                    if pending and ti - pending[0][0] >= NORM_DELAY:
                        _, ph, pd = pending.pop(0)
                        flush_norm(ph, pd)
                    yield
                for _, ph, pd in pending:
                    flush_norm(ph, pd)

            def outproj_steps(qc, ctxq):
                """Generator emitting the out-projection for q-chunk qc one
                (dc, st) group at a time; interleaved into the NEXT q-chunk's
                attention stream so its Wo DMAs and eviction chains hide."""
                wots = []

                def wot_fetch(dc):
                    wot = wopool.tile([P, HPG, NCHUNK], BF16, tag="wot",
                                      name=f"wot{qc}_{dc}")
                    nc.sync.dma_start(
                        wot[:], wo_v[:, :, dc * NCHUNK:(dc + 1) * NCHUNK])
                    wots.append(wot)

                wot_fetch(0)
                wot_fetch(1)
                for dc in range(SC):
                    if dc + 2 < SC:
                        wot_fetch(dc + 2)
                    wot = wots[dc]
                    for st in range(4):
                        stq = qc * 4 + st
                        ops = bank()
                        for h in range(HPG):
                            nc.tensor.matmul(
                                ops[:], ctxq[:, h, st * P:(st + 1) * P],
                                wot[:, h, :],
                                start=(h == 0), stop=(h == HPG - 1))
                        osb = opool.tile([P, NCHUNK], BF16, tag="osb")
                        if st % 4 == 3:
                            nc.scalar.copy(osb[:], ops[:])
                        else:
                            nc.vector.tensor_copy(osb[:], ops[:])
                        nc.sync.dma_start(
                            out_v[stq, :, dc * NCHUNK:(dc + 1) * NCHUNK],
                            osb[:])
                        yield

            # Interleave: s-chunk tails (rope/transpose, PE+DVE) are emitted
            # after the next s-chunk's first projection matmuls so the PE
            # queue never drains while evictions/ropes complete.
            _qacc = {}
            _kvacc = {}
            kv_tail = None
            for sc in range(SC):
                if sc + 1 < SC:
                    stage_x(sc + 1, [(0, 8), (8, DT)], "xg", 4)
                _qacc[sc] = [bank() for _ in range(HPG)]
                q_proj(sc, range(0, 2))
                if kv_tail is not None:
                    for _ in kv_tail_steps(*kv_tail):
                        pass
                q_proj(sc, range(2, DT))
                qsbs = q_evict(sc)
                _kvacc[sc] = (bank(), bank())
                kv_proj(sc, range(0, 2))
                for _ in q_tail_steps(sc, qsbs):
                    pass
                kv_proj(sc, range(2, DT))
                kv_tail = (sc,) + tuple(kv_evict(sc))

            # Last chunk's K-rope/V-transpose runs inline at phase A end (its
            # kf/vnat are only read by q-chunk 3; the DVE work hides under
            # the first attention units).
            for _ in kv_tail_steps(*kv_tail):
                pass
            out_gen = None
            for qc in range(SC):
                ctxq = cpool.tile([P, HPG, NCHUNK], BF16, tag="ctxq",
                                  name=f"ctxq{qc}")
                n_steps = 4 * (2 * qc + 4) + LOOKAHEAD
                start_at = max(2, n_steps // 3)
                k = 0
                for _ in attention_steps(qc, ctxq):
                    k += 1
                    if out_gen is not None and k >= start_at:
                        next(out_gen, None)
                if out_gen is not None:
                    for _ in out_gen:
                        pass
                out_gen = outproj_steps(qc, ctxq)
            for _ in out_gen:
                pass

    nc.compile()
    return nc


def _host_consts():
    i = np.arange(0, HD, 2, dtype=np.float32)
    inv = (1.0 / (10000.0 ** (i / HD))).astype(np.float32)      # [64]
    t = np.arange(S, dtype=np.float32)
    freqs = t[:, None] * inv[None, :]                           # [S, 64] f32
    emb = np.concatenate([freqs, freqs], axis=1)                # [S, 128]
    cosT = np.cos(emb).T.astype(NPBF16).copy()                  # [128, S]
    sinT = np.sin(emb).T.astype(np.float32).copy()
    prot = np.zeros((HD, HD), dtype=np.float32)
    half = HD // 2
    for ii in range(half):
        prot[ii + half, ii] = -1.0     # rot[i] = -x[i+64], i < 64
    for ii in range(half, HD):
        prot[ii - half, ii] = 1.0      # rot[i] =  x[i-64], i >= 64
    return cosT, sinT, prot.astype(NPBF16)


def _in_maps(x, Wq, Wk, Wv, Wo):
    cosT, sinT, prot = _host_consts()
    # shared per-batch / per-group shards (read-only, safe to alias
    # across the in_maps of the 4 cores that use them)
    xTs = [np.ascontiguousarray(x[b].T).astype(NPBF16) for b in range(B)]
    wqs = [np.ascontiguousarray(Wq[:, g * GD:(g + 1) * GD]).astype(NPBF16)
           for g in range(G)]
    wks = [np.ascontiguousarray(Wk[:, g * HD:(g + 1) * HD]).astype(NPBF16)
           for g in range(G)]
    wvs = [np.ascontiguousarray(Wv[:, g * HD:(g + 1) * HD]).astype(NPBF16)
           for g in range(G)]
    wos = [np.ascontiguousarray(Wo[g * GD:(g + 1) * GD, :]).astype(NPBF16)
           for g in range(G)]
    maps = []
    for c in range(8):
        b, g = c // 4, c % 4
        maps.append({
            "xT": xTs[b], "wq": wqs[g], "wk": wks[g], "wv": wvs[g],
            "wo": wos[g], "cosT": cosT, "sinT": sinT, "prot": prot,
        })
    return maps


def run(x, Wq, Wk, Wv, Wo, trace=False, **trace_kw):
    if "nc" not in _CACHE:
        _CACHE["nc"] = _build()
    nc = _CACHE["nc"]
    maps = _in_maps(
        np.asarray(x, dtype=np.float32), np.asarray(Wq, dtype=np.float32),
        np.asarray(Wk, dtype=np.float32), np.asarray(Wv, dtype=np.float32),
        np.asarray(Wo, dtype=np.float32))
    res = run_bass_kernel_spmd(
        nc, maps, core_ids=list(range(8)), trace=trace, **trace_kw)
    parts = [res.results[c]["out"].astype(np.float32) for c in range(8)]
    full = np.stack([
        parts[0] + parts[1] + parts[2] + parts[3],
        parts[4] + parts[5] + parts[6] + parts[7],
    ]).astype(np.float32)
    return full, res


def kernel(x, Wq, Wk, Wv, Wo, mask=None):
    full, _ = run(x, Wq, Wk, Wv, Wo, trace=False)
    return full


# revision 34
# speedup vs baseline: 1.1415x; 1.1415x over previous
"""GQA MultiHeadAttention (RoPE, causal) Bass/Tile kernel for 8 Trainium2 cores.

Problem: x[2,2048,2048] @ Wq/Wk/Wv -> RoPE -> causal GQA attention -> @ Wo.
D=2048, H=16 heads, G=4 KV groups, HD=128, B=2, S=2048.

Sharding (SPMD, one program, per-core data):
  core c -> batch b=c//4, KV-group g=c%4 (heads 4g..4g+3).
  Each core: QKV projection for its group from x[b]^T, RoPE, 4 heads of
  attention, and a row-shard of the output projection (Wo rows for its
  heads) producing a partial [2048,2048] output. Host sums the 4 partials
  per batch.

On-chip layouts are all "transposed" (feature dim on partitions):
  QT/KT/VT [hd, s]; scores computed as scoresT [k, q]; ctxT [hd, q];
  out-projection uses ctxT slices as stationary to produce natural [s, d].

v2 changes vs the 364us baseline:
  - bf16 on the whole matmul/DMA path (PSUM accumulate stays f32);
    halves HBM traffic and SBUF footprint.
  - softmax denominator off the PE: DVE accumulates exp tiles per head,
    GpSimd partition_all_reduce gives the broadcast denominator, DVE
    reciprocal+scale evicts ctx. Removes 160 PE den matmuls and the 16
    broadcast matmuls (and their head-boundary PE stalls).
  - diagonal score tiles compute only the valid q-range (moving operand
    shrinks 512->128..512); causal mask applies to one 128x128 subblock.
  - off-diagonal k-tiles run first within each head so diagonal partial
    regions accumulate onto an initialized PSUM bank.
"""

import sys

if "/opt/trn_rl_repo" not in sys.path:
    sys.path.insert(0, "/opt/trn_rl_repo")

from contextlib import ExitStack

import numpy as np
import ml_dtypes

import concourse.bass as bass
import concourse.tile as tile
from concourse import bacc, bass_isa, mybir
from concourse.bass_utils import run_bass_kernel_spmd
from concourse.masks import make_identity

F32 = mybir.dt.float32
BF16 = mybir.dt.bfloat16
AF = mybir.ActivationFunctionType
NPBF16 = np.dtype(ml_dtypes.bfloat16)

B, S, D = 2, 2048, 2048
H, G, HD = 16, 4, 128
HPG = H // G          # heads per group = 4
GD = HPG * HD         # group width = 512
P = 128
NCHUNK = 512          # matmul moving free dim
SC = S // NCHUNK      # 4 s-chunks
DT = D // P           # 16 d-tiles
ST = S // P           # 16 s-tiles
SCALE = 1.0 / float(np.sqrt(HD))

_CACHE = {}


def _build():
    nc = bacc.Bacc("TRN2", target_bir_lowering=False, debug=False, num_devices=8)

    # ---- DRAM I/O (per-core shards, bf16 on the wire) ----
    xT = nc.dram_tensor("xT", [D, S], BF16, kind="ExternalInput").ap()
    wq = nc.dram_tensor("wq", [D, GD], BF16, kind="ExternalInput").ap()
    wk = nc.dram_tensor("wk", [D, HD], BF16, kind="ExternalInput").ap()
    wv = nc.dram_tensor("wv", [D, HD], BF16, kind="ExternalInput").ap()
    wo = nc.dram_tensor("wo", [GD, D], BF16, kind="ExternalInput").ap()
    cosT = nc.dram_tensor("cosT", [HD, S], BF16, kind="ExternalInput").ap()
    sinT = nc.dram_tensor("sinT", [HD, S], F32, kind="ExternalInput").ap()
    prot = nc.dram_tensor("prot", [HD, HD], BF16, kind="ExternalInput").ap()
    out = nc.dram_tensor("out", [S, D], BF16, kind="ExternalOutput").ap()

    xT_v = xT.rearrange("(t p) s -> p t s", p=P)          # [128, 16, 2048]
    wq_v = wq.rearrange("(t p) o -> p t o", p=P)          # [128, 16, 512]
    wk_v = wk.rearrange("(t p) o -> p t o", p=P)          # [128, 16, 128]
    wv_v = wv.rearrange("(t p) o -> p t o", p=P)
    wo_v = wo.rearrange("(h p) d -> p h d", p=P)          # [128, 4, 2048]
    out_v = out.rearrange("(t p) d -> t p d", p=P)        # [16, 128, 2048]

    with tile.TileContext(nc) as tc:
        with ExitStack() as ctx:
            pers = ctx.enter_context(tc.tile_pool(name="pers", bufs=1))
            psum = ctx.enter_context(tc.tile_pool(name="psum", bufs=6, space="PSUM"))
            xpool = ctx.enter_context(tc.tile_pool(name="xpool", bufs=10))
            spool = ctx.enter_context(tc.tile_pool(name="spool", bufs=4))
            epool = ctx.enter_context(tc.tile_pool(name="epool", bufs=8))
            dpool = ctx.enter_context(tc.tile_pool(name="dpool", bufs=3))
            wopool = ctx.enter_context(tc.tile_pool(name="wopool", bufs=3))
            cpool = ctx.enter_context(tc.tile_pool(name="cpool", bufs=2))
            cspool = ctx.enter_context(tc.tile_pool(name="cspool", bufs=3))
            evpool = ctx.enter_context(tc.tile_pool(name="evpool", bufs=6))
            opool = ctx.enter_context(tc.tile_pool(name="opool", bufs=4))

            _bank_n = [0]

            def bank():
                _bank_n[0] += 1
                return psum.tile([P, NCHUNK], F32, tag="bank",
                                 name=f"bank{_bank_n[0]}")

            # ---- persistent tiles ----
            wq_t = pers.tile([P, DT, GD], BF16, tag="wq")
            wk_t = pers.tile([P, DT, HD], BF16, tag="wk")
            wv_t = pers.tile([P, DT, HD], BF16, tag="wv")
            cos_t = pers.tile([P, S], BF16, tag="cos")
            sin_t = pers.tile([P, S], F32, tag="sin")
            prot_t = pers.tile([P, HD], BF16, tag="prot")
            ident = pers.tile([P, P], BF16, tag="ident")
            masksb = pers.tile([P, P], BF16, tag="masksb")    # 0 / -1e4 tri
            onescol = pers.tile([P, 1], BF16, tag="onescol")
            onesrow = pers.tile([1, P], BF16, tag="onesrow")
            qf = pers.tile([P, HPG, S], BF16, tag="qf")       # roped Q^T, 4 heads
            kf = pers.tile([P, S], BF16, tag="kf")            # roped K^T
            vnat = pers.tile([P, ST, HD], BF16, tag="vnat")   # V natural [s, hd]

            # Batched x loads: 2-dt pieces for the first s-chunk (so the
            # first matmuls start after ~0.6 MB of DMA), whole 8-dt groups
            # prefetched one chunk ahead for the rest. Few dma_starts keeps
            # the SP sequencer's per-issue DGE overhead off the critical path.
            xgroups = {}

            def stage_x(sc, pieces, tag, bufs):
                s0 = sc * NCHUNK
                lst = []
                for d0, d1 in pieces:
                    xg = xpool.tile([P, d1 - d0, NCHUNK], BF16, tag=tag,
                                    bufs=bufs, name=f"xg{sc}_{d0}")
                    nc.sync.dma_start(xg[:], xT_v[:, d0:d1, s0:s0 + NCHUNK])
                    lst.append((d0, d1, xg))
                xgroups.setdefault(sc, []).extend(lst)

            def xslice(sc, dt):
                for d0, d1, xg in xgroups[sc]:
                    if d0 <= dt < d1:
                        return xg[:, dt - d0, :]
                raise KeyError((sc, dt))

            # chunk-0 x pieces interleaved with per-piece weight loads;
            # the first two pieces are single-dt so the first matmul starts
            # after ~0.3 MB of DMA
            pieces0 = [(0, 1), (1, 2)] + [(d, d + 2) for d in range(2, DT, 2)]
            for d0, d1 in pieces0:
                nc.sync.dma_start(wq_t[:, d0:d1, :], wq_v[:, d0:d1, :])
                stage_x(0, [(d0, d1)], "xg0", 9)
            nc.sync.dma_start(wk_t[:], wk_v[:])
            nc.sync.dma_start(wv_t[:], wv_v[:])
            nc.sync.dma_start(cos_t[:], cosT[:])
            nc.sync.dma_start(sin_t[:], sinT[:])
            nc.sync.dma_start(prot_t[:], prot[:])
            make_identity(nc, ident[:])
            # additive causal mask for diagonal 128x128 subblocks:
            # masksb[r, c] = 0 where c >= r (keep), -1e4 where c < r, so a
            # single PE matmul (stat=ident) accumulates it into scores PSUM
            # and exp underflows the masked slots to 0. Keeps GpSimd out of
            # the scores->exp->ctx chain entirely.
            nc.gpsimd.memset(onescol[:], 1.0)
            nc.gpsimd.memset(onesrow[:], 1.0)
            nc.gpsimd.memset(masksb[:], 0.0)
            nc.gpsimd.affine_select(
                out=masksb[:], in_=masksb[:],
                compare_op=mybir.AluOpType.is_ge,
                fill=-1.0e4, base=0, channel_multiplier=-1,
                pattern=[[1, P]],
            )

            # ================= Phase A: QKV projection + RoPE + V^T -> V ====
            def rope(dst, src_sb, sc):
                """dst[128,512] bf16 slice = rope(src_sb [128,512] bf16)."""
                cs = cos_t[:, sc * NCHUNK:(sc + 1) * NCHUNK]
                sn = sin_t[:, sc * NCHUNK:(sc + 1) * NCHUNK]
                rotps = bank()
                nc.tensor.matmul(rotps[:], prot_t[:], src_sb, start=True, stop=True)
                t1 = spool.tile([P, NCHUNK], BF16, tag="t1")
                nc.vector.tensor_mul(t1[:], rotps[:], sn)
                nc.vector.tensor_mul(dst, src_sb, cs)
                nc.vector.tensor_add(dst, dst, t1[:])

            def q_proj(sc, dts):
                """Q projection matmuls for s-chunk sc over d-tiles dts."""
                for dt in dts:
                    xt = xslice(sc, dt)
                    for h in range(HPG):
                        nc.tensor.matmul(
                            _qacc[sc][h][:], wq_t[:, dt, h * HD:(h + 1) * HD],
                            xt, start=dt == 0, stop=dt == DT - 1)

            def kv_proj(sc, dts):
                kps, vps = _kvacc[sc]
                for dt in dts:
                    xt = xslice(sc, dt)
                    nc.tensor.matmul(kps[:], wk_t[:, dt, :], xt,
                                     start=dt == 0, stop=dt == DT - 1)
                    nc.tensor.matmul(vps[:], wv_t[:, dt, :], xt,
                                     start=dt == 0, stop=dt == DT - 1)

            def q_evict(sc):
                """Q PSUM -> SBUF (frees the 4 Q banks before the KV pass)."""
                sbs = []
                for h in range(HPG):
                    qsb = evpool.tile([P, NCHUNK], BF16, tag="ev",
                                      name=f"qsb{sc}_{h}")
                    if h % 2 == 0:
                        nc.scalar.copy(qsb[:], _qacc[sc][h][:])
                    else:
                        nc.vector.tensor_copy(qsb[:], _qacc[sc][h][:])
                    sbs.append(qsb[:])
                return sbs

            def q_tail_steps(sc, sbs):
                """Q RoPE; interleaved into the same chunk's KV pass."""
                s0 = sc * NCHUNK
                for h in range(HPG):
                    rope(qf[:, h, s0:s0 + NCHUNK], sbs[h], sc)
                    yield

            def kv_evict(sc):
                kps, vps = _kvacc[sc]
                ksb = evpool.tile([P, NCHUNK], BF16, tag="ev", name=f"ksb{sc}")
                nc.vector.tensor_copy(ksb[:], kps[:])
                vsb = evpool.tile([P, NCHUNK], BF16, tag="ev", name=f"vsb{sc}")
                nc.scalar.copy(vsb[:], vps[:])
                return ksb, vsb

            def kv_tail_steps(sc, ksb, vsb):
                """K RoPE + V^T transpose; interleaved into the next chunk's
                Q pass (or, for the last chunk, into q-chunk 0's attention —
                kf/vnat of chunk sc are only read by q-chunks >= sc)."""
                rope(kf[:, sc * NCHUNK:(sc + 1) * NCHUNK], ksb[:], sc)
                yield
                for j in range(4):
                    _bank_n[0] += 1
                    tps = psum.tile([P, P], BF16, tag="bank",
                                    name=f"tbank{_bank_n[0]}")
                    nc.tensor.transpose(
                        tps[:], vsb[:, j * P:(j + 1) * P], ident[:])
                    nc.vector.tensor_copy(vnat[:, sc * 4 + j, :], tps[:])
                    yield

            # ---- Phase B/C: attention per (q-chunk, head) + out-projection
            LOOKAHEAD = 6     # tiles of scores->ctx pipeline lag
            NORM_DELAY = 8    # tiles after head end before recip/mul emission

            def attention_steps(qc, ctxq):
                """Generator emitting attention for q-chunk qc one score-unit
                at a time. Off-diagonal k-tiles are processed in PAIRS that
                share a 2-bank PSUM tile and a single exp instruction
                (amortizes the Activation engine's per-op overhead — Act is
                the throughput limiter of this phase). Diagonal tiles run
                singly with the moving operand shrunk to the valid q-range
                and the additive -1e4 triangle matmul'd into scores PSUM.
                The denominator accumulates on DVE into a 2-lane accumulator;
                at head end the ctx bank is evicted to SBUF immediately
                (Scalar) and the reciprocal/normalize (DVE/GpSimd) is
                DEFERRED a few tiles so the partition_all_reduce latency
                (~4us on HW) never head-of-line-blocks either engine."""
                q0 = qc * NCHUNK
                base = 4 * qc
                nki = base + 4
                units = []
                for h in range(HPG):
                    for i in range(0, base, 2):
                        units.append((h, (i, i + 2)))      # off-diag pair
                    for ki in range(base, nki):
                        units.append((h, (ki, ki + 1)))    # diag single
                tile_seq = [(h, ki) for h, u in units for ki in range(*u)]
                ets = {}
                acc2s = {}
                ctxbanks = {}
                csbs = {}
                pending = []   # (tile_index_at_emit, h, denb)

                def qr0_of(ki):
                    return (ki - base) * P if ki >= base else 0

                def do_unit(h, u):
                    a, b = u
                    if b - a == 2:
                        _bank_n[0] += 1
                        sps = psum.tile([P, 2, NCHUNK], F32, tag="bank2",
                                        bufs=1, name=f"b2_{_bank_n[0]}")
                        for j in range(2):
                            nc.tensor.matmul(
                                sps[:, j, :], kf[:, (a + j) * P:(a + j + 1) * P],
                                qf[:, h, q0:q0 + NCHUNK], start=True, stop=True)
                        et = epool.tile([P, 2, NCHUNK], BF16, tag="et2",
                                        bufs=5, name=f"et2_{qc}_{h}_{a}")
                        nc.scalar.activation(et[:], sps[:], AF.Exp, scale=SCALE)
                        if a == 0:
                            acc = dpool.tile([P, 2, NCHUNK], BF16, tag="acc",
                                             name=f"acc{qc}_{h}")
                            acc2s[h] = acc
                            nc.vector.tensor_copy(acc[:], et[:])
                        else:
                            acc = acc2s[h]
                            nc.vector.tensor_add(acc[:], acc[:], et[:])
                        ets[(h, a)] = et[:, 0, :]
                        ets[(h, a + 1)] = et[:, 1, :]
                    else:
                        ki = a
                        qr0 = qr0_of(ki)
                        sps = bank()
                        nc.tensor.matmul(
                            sps[:, qr0:NCHUNK], kf[:, ki * P:(ki + 1) * P],
                            qf[:, h, q0 + qr0:q0 + NCHUNK],
                            start=True, stop=False)
                        nc.tensor.matmul(
                            sps[:, qr0:qr0 + P], ident[:], masksb[:],
                            start=False, stop=True)
                        et = epool.tile([P, NCHUNK], BF16, tag="et",
                                        bufs=6, name=f"et{qc}_{h}_{ki}")
                        nc.scalar.activation(et[:, qr0:NCHUNK],
                                             sps[:, qr0:NCHUNK],
                                             AF.Exp, scale=SCALE)
                        if ki == 0:
                            acc = dpool.tile([P, 2, NCHUNK], BF16, tag="acc",
                                             name=f"acc{qc}_{h}")
                            acc2s[h] = acc
                            nc.vector.tensor_copy(acc[:, 0, :], et[:])
                            nc.vector.memset(acc[:, 1, :], 0.0)
                        else:
                            acc = acc2s[h]
                            nc.vector.tensor_add(
                                acc[:, ki & 1, qr0:NCHUNK],
                                acc[:, ki & 1, qr0:NCHUNK], et[:, qr0:NCHUNK])
                        ets[(h, ki)] = et[:, qr0:NCHUNK]

                def do_ctx(h, ki):
                    qr0 = qr0_of(ki)
                    if ki == 0:
                        ctxbanks[h] = bank()
                    nc.tensor.matmul(ctxbanks[h][:, qr0:NCHUNK],
                                     vnat[:, ki, :], ets.pop((h, ki)),
                                     start=(ki == 0), stop=(ki == nki - 1))
                    if ki == nki - 1:
                        csb = cspool.tile([P, NCHUNK], F32, tag="ctxsb",
                                          name=f"csb{qc}_{h}")
                        nc.scalar.copy(csb[:], ctxbanks[h][:])
                        csbs[h] = csb
                        acc = acc2s.pop(h)
                        den1 = spool.tile([P, NCHUNK], BF16, tag="den1")
                        nc.vector.tensor_add(den1[:], acc[:, 0, :],
                                             acc[:, 1, :])
                        del ctxbanks[h]
                        if qc == SC - 1 and h == HPG - 1:
                            # final head: norm via PE ones-matmuls (~3us
                            # faster than partition_all_reduce and keeps the
                            # DVE queue clear for the bare out-projection)
                            dps = bank()
                            nc.tensor.matmul(dps[0:1, :], onescol[:],
                                             den1[:], start=True, stop=True)
                            rec32 = spool.tile([1, NCHUNK], F32, tag="rec32")
                            nc.vector.reciprocal_approx_fast(rec32[:],
                                                             dps[0:1, :])
                            rec16 = spool.tile([1, NCHUNK], BF16, tag="rec16")
                            nc.vector.tensor_copy(rec16[:], rec32[:])
                            bps = bank()
                            nc.tensor.matmul(bps[:], onesrow[:], rec16[:],
                                             start=True, stop=True)
                            nc.vector.tensor_mul(ctxq[:, h, :],
                                                 csbs.pop(h)[:], bps[:])
                            return None
                        denb = spool.tile([P, NCHUNK], F32, tag="denb")
                        nc.gpsimd.partition_all_reduce(
                            denb[:], den1[:], channels=P,
                            reduce_op=bass_isa.ReduceOp.add)
                        return denb
                    return None

                def flush_norm(h, denb):
                    recb = spool.tile([P, NCHUNK], F32, tag="recb")
                    nc.vector.reciprocal_approx_fast(recb[:], denb[:])
                    nc.vector.tensor_mul(ctxq[:, h, :], csbs.pop(h)[:],
                                         recb[:])

                ti = 0
                emitted = 0
                for h, u in units:
                    do_unit(h, u)
                    emitted += u[1] - u[0]
                    while ti < emitted - LOOKAHEAD:
                        hh, kk = tile_seq[ti]
                        denb = do_ctx(hh, kk)
                        if denb is not None:
                            pending.append((ti, hh, denb))
                        ti += 1
                        if pending and ti - pending[0][0] >= NORM_DELAY:
                            _, ph, pd = pending.pop(0)
                            flush_norm(ph, pd)
                    yield
                while ti < len(tile_seq):
                    hh, kk = tile_seq[ti]
                    denb = do_ctx(hh, kk)
                    if denb is not None:
                        pending.append((ti, hh, denb))
                    ti += 1
                    if pending and ti - pending[0][0] >= NORM_DELAY:
                        _, ph, pd = pending.pop(0)
                        flush_norm(ph, pd)
                    yield
                for _, ph, pd in pending:
                    flush_norm(ph, pd)

            def outproj_steps(qc, ctxq):
                """Eagerly fetch the first two Wo chunks at CALL time (a
                plain generator would defer them to the first next(), which
                for the last q-chunk is after all attention ends — putting
                the DMA wait on the bare tail), then return the generator."""
                wots = []

                def wot_fetch(dc):
                    wot = wopool.tile([P, HPG, NCHUNK], BF16, tag="wot",
                                      name=f"wot{qc}_{dc}")
                    nc.sync.dma_start(
                        wot[:], wo_v[:, :, dc * NCHUNK:(dc + 1) * NCHUNK])
                    wots.append(wot)

                wot_fetch(0)
                wot_fetch(1)
                return _outproj_gen(qc, ctxq, wots, wot_fetch)

            def _outproj_gen(qc, ctxq, wots, wot_fetch):
                for dc in range(SC):
                    if dc + 2 < SC:
                        wot_fetch(dc + 2)
                    wot = wots[dc]
                    for st in range(4):
                        stq = qc * 4 + st
                        ops = bank()
                        for h in range(HPG):
                            nc.tensor.matmul(
                                ops[:], ctxq[:, h, st * P:(st + 1) * P],
                                wot[:, h, :],
                                start=(h == 0), stop=(h == HPG - 1))
                        osb = opool.tile([P, NCHUNK], BF16, tag="osb")
                        if st % 4 == 3:
                            nc.scalar.copy(osb[:], ops[:])
                        else:
                            nc.vector.tensor_copy(osb[:], ops[:])
                        nc.sync.dma_start(
                            out_v[stq, :, dc * NCHUNK:(dc + 1) * NCHUNK],
                            osb[:])
                        yield

            # Interleave: s-chunk tails (rope/transpose, PE+DVE) are emitted
            # after the next s-chunk's first projection matmuls so the PE
            # queue never drains while evictions/ropes complete.
            _qacc = {}
            _kvacc = {}
            kv_tail = None
            for sc in range(SC):
                if sc + 1 < SC:
                    stage_x(sc + 1, [(0, 8), (8, DT)], "xg", 4)
                _qacc[sc] = [bank() for _ in range(HPG)]
                q_proj(sc, range(0, 2))
                if kv_tail is not None:
                    for _ in kv_tail_steps(*kv_tail):
                        pass
                q_proj(sc, range(2, DT))
                qsbs = q_evict(sc)
                _kvacc[sc] = (bank(), bank())
                kv_proj(sc, range(0, 2))
                for _ in q_tail_steps(sc, qsbs):
                    pass
                kv_proj(sc, range(2, DT))
                kv_tail = (sc,) + tuple(kv_evict(sc))

            # Last chunk's K-rope/V-transpose runs inline at phase A end (its
            # kf/vnat are only read by q-chunk 3; the DVE work hides under
            # the first attention units).
            for _ in kv_tail_steps(*kv_tail):
                pass
            out_gen = None
            for qc in range(SC):
                ctxq = cpool.tile([P, HPG, NCHUNK], BF16, tag="ctxq",
                                  name=f"ctxq{qc}")
                n_steps = 4 * (2 * qc + 4) + LOOKAHEAD
                start_at = max(2, n_steps // 3)
                k = 0
                for _ in attention_steps(qc, ctxq):
                    k += 1
                    if out_gen is not None and k >= start_at:
                        next(out_gen, None)
                if out_gen is not None:
                    for _ in out_gen:
                        pass
                out_gen = outproj_steps(qc, ctxq)
            for _ in out_gen:
                pass

    nc.compile()
    return nc


def _host_consts():
    i = np.arange(0, HD, 2, dtype=np.float32)
    inv = (1.0 / (10000.0 ** (i / HD))).astype(np.float32)      # [64]
    t = np.arange(S, dtype=np.float32)
    freqs = t[:, None] * inv[None, :]                           # [S, 64] f32
    emb = np.concatenate([freqs, freqs], axis=1)                # [S, 128]
    cosT = np.cos(emb).T.astype(NPBF16).copy()                  # [128, S]
    sinT = np.sin(emb).T.astype(np.float32).copy()
    prot = np.zeros((HD, HD), dtype=np.float32)
    half = HD // 2
    for ii in range(half):
        prot[ii + half, ii] = -1.0     # rot[i] = -x[i+64], i < 64
    for ii in range(half, HD):
        prot[ii - half, ii] = 1.0      # rot[i] =  x[i-64], i >= 64
    return cosT, sinT, prot.astype(NPBF16)


def _in_maps(x, Wq, Wk, Wv, Wo):
    cosT, sinT, prot = _host_consts()
    # shared per-batch / per-group shards (read-only, safe to alias
    # across the in_maps of the 4 cores that use them)
    xTs = [np.ascontiguousarray(x[b].T).astype(NPBF16) for b in range(B)]
    wqs = [np.ascontiguousarray(Wq[:, g * GD:(g + 1) * GD]).astype(NPBF16)
           for g in range(G)]
    wks = [np.ascontiguousarray(Wk[:, g * HD:(g + 1) * HD]).astype(NPBF16)
           for g in range(G)]
    wvs = [np.ascontiguousarray(Wv[:, g * HD:(g + 1) * HD]).astype(NPBF16)
           for g in range(G)]
    wos = [np.ascontiguousarray(Wo[g * GD:(g + 1) * GD, :]).astype(NPBF16)
           for g in range(G)]
    maps = []
    for c in range(8):
        b, g = c // 4, c % 4
        maps.append({
            "xT": xTs[b], "wq": wqs[g], "wk": wks[g], "wv": wvs[g],
            "wo": wos[g], "cosT": cosT, "sinT": sinT, "prot": prot,
        })
    return maps


def run(x, Wq, Wk, Wv, Wo, trace=False, **trace_kw):
    if "nc" not in _CACHE:
        _CACHE["nc"] = _build()
    nc = _CACHE["nc"]
    maps = _in_maps(
        np.asarray(x, dtype=np.float32), np.asarray(Wq, dtype=np.float32),
        np.asarray(Wk, dtype=np.float32), np.asarray(Wv, dtype=np.float32),
        np.asarray(Wo, dtype=np.float32))
    res = run_bass_kernel_spmd(
        nc, maps, core_ids=list(range(8)), trace=trace, **trace_kw)
    parts = [res.results[c]["out"].astype(np.float32) for c in range(8)]
    full = np.stack([
        parts[0] + parts[1] + parts[2] + parts[3],
        parts[4] + parts[5] + parts[6] + parts[7],
    ]).astype(np.float32)
    return full, res


def kernel(x, Wq, Wk, Wv, Wo, mask=None):
    full, _ = run(x, Wq, Wk, Wv, Wo, trace=False)
    return full
